# revision 27
# baseline (speedup 1.0000x reference)
"""Multi-head causal attention on 8 Trainium2 NeuronCores.

Problem: x [2, 2048, 1024] f32; Wq/Wk/Wv [1024, 1024]; Wo [1024, 1024]; bo [1024].
  q/k/v = split_heads(x @ W*)  (16 heads, head_dim 64)
  scores = q k^T, causal mask, / sqrt(1024), softmax, out = (w v) @ Wo + bo

Sharding: tensor-parallel over heads, 8-way (Megatron-style): core c computes
heads {2c, 2c+1} for BOTH batches. The concat+out_proj needs all heads, so
cores exchange attention outputs with a single 8-rank AllToAll (each core
sends, per destination core j, its two heads' attn^T restricted to j's output
row-slice). After the exchange core j holds attn^T [1024, 512] for
(batch j//4, rows 512*(j%4):+512), applies the full out_proj + bias, and
returns its 512-row slice of the output; the host reassembles.

On-chip layout trick: attention is computed fully transposed
(scores^T = K Q^T in [k, q] layout) so the softmax weights come out exactly in
the layout the attn-value matmul wants as its moving operand, and the AV
result comes out as attn^T [d, q] which is exactly the stationary layout
out_proj wants. Zero on-chip transposes in the attention path. The softmax
denominator is obtained for free by augmenting V with a ones-column (row 64 of
the AV psum accumulates sum(w)).

Compute dtype bf16 (fp32 accumulation in PSUM).
"""

from contextlib import ExitStack

import numpy as np

import concourse.bass as bass
import concourse.tile as tile
from concourse import bacc, mybir
from concourse.bass_utils import run_bass_kernel_spmd
from concourse.masks import make_identity

F32 = mybir.dt.float32
BF16 = mybir.dt.bfloat16

N_CORES = 8
B = 2
S = 2048
D = 1024
H = 16
DH = 64
H_PER = 2              # heads per core
DCOL = H_PER * DH      # 128: projection output cols per core
KT = D // 128          # 8 contraction tiles
SB = S // 128          # 16 sequence blocks
NQS = S // 512         # 4 q-spans
S_SLICE = S // 4       # 512 output rows per core
SCALE = 1.0 / np.sqrt(np.float32(D))

_CACHE = {}


def build():
    """Build the SPMD program (identical on all 8 cores)."""
    nc = bacc.Bacc("TRN2", target_bir_lowering=False, debug=False)

    x_t = nc.dram_tensor("x", [B, S, D], F32, kind="ExternalInput")
    wq_t = nc.dram_tensor("wq", [D, DCOL], F32, kind="ExternalInput")
    wk_t = nc.dram_tensor("wk", [D, DCOL], F32, kind="ExternalInput")
    wv_t = nc.dram_tensor("wv", [D, DCOL], F32, kind="ExternalInput")
    wo_t = nc.dram_tensor("wo", [D, D], F32, kind="ExternalInput")
    bo_t = nc.dram_tensor("bo", [1, D], F32, kind="ExternalInput")
    out_t = nc.dram_tensor("out", [S_SLICE, D], F32, kind="ExternalOutput")

    # collective buffers (internal DRAM); the attn^T exchange is split per
    # local head so the first A2A overlaps the second half of attention.
    warm_in = nc.dram_tensor("warm_in", [8, 16], F32)
    warm_out = nc.dram_tensor("warm_out", [8, 16], F32)
    a2a_in = [
        nc.dram_tensor(f"a2a_in{h}", [8, DH, 512], BF16) for h in range(H_PER)
    ]
    a2a_out = [
        nc.dram_tensor(f"a2a_out{h}", [8, DH, 512], BF16) for h in range(H_PER)
    ]

    with tile.TileContext(nc) as tc, ExitStack() as ctx:
        const = ctx.enter_context(tc.tile_pool(name="const", bufs=1))
        persist = ctx.enter_context(tc.tile_pool(name="persist", bufs=1))
        stage = ctx.enter_context(tc.tile_pool(name="stage", bufs=3))
        wstage = ctx.enter_context(tc.tile_pool(name="wstage", bufs=2))
        wpool = ctx.enter_context(tc.tile_pool(name="wpool", bufs=4))
        rbpool = ctx.enter_context(tc.tile_pool(name="rbpool", bufs=3))
        spool = ctx.enter_context(tc.tile_pool(name="spool", bufs=4))
        opool = ctx.enter_context(tc.tile_pool(name="opool", bufs=3))
        ps_mm = ctx.enter_context(tc.tile_pool(name="ps_mm", bufs=2, space="PSUM"))
        ps_o = ctx.enter_context(tc.tile_pool(name="ps_o", bufs=2, space="PSUM"))
        ps_t = ctx.enter_context(tc.tile_pool(name="ps_t", bufs=2, space="PSUM"))

        # ---- warmup collective: absorbs the per-execution ncfw entry cost
        # concurrently with the compute phase.
        nc.gpsimd.collective_compute(
            "AllToAll", mybir.AluOpType.bypass,
            replica_groups=[list(range(8))],
            ins=[warm_in.ap().opt()], outs=[warm_out.ap().opt()],
        )

        identity = const.tile([128, 128], BF16)
        make_identity(nc, identity)

        attnT = [
            persist.tile([128, S], BF16, tag=f"attnT{b}", name=f"attnT{b}")
            for b in range(B)
        ]

        # ---- phase helpers -------------------------------------------------
        def load_transpose_x(b, xT):
            """x[b] -> x^T bf16 [128, KT, S] via PE transpose."""
            for sb in range(SB):
                xn = stage.tile([128, D], F32, tag="xn", name="xn")
                nc.sync.dma_start(out=xn, in_=x_t[b, sb * 128:(sb + 1) * 128, :])
                xb = stage.tile([128, D], BF16, tag="xb", name="xb")
                nc.vector.tensor_copy(out=xb, in_=xn)
                for g in range(2):  # 2 groups of 4 d-blocks -> one psum bank
                    pt = ps_t.tile([128, 4, 128], BF16, tag="pt", name="pt")
                    for k in range(4):
                        kt = g * 4 + k
                        nc.tensor.transpose(
                            pt[:, k, :], xb[:, kt * 128:(kt + 1) * 128], identity)
                    dst = xT[:, g * 4:(g + 1) * 4, sb * 128:(sb + 1) * 128]
                    nc.vector.tensor_copy(out=dst, in_=pt)

        def load_weights():
            def load_cast(dram_ap, kt_cols, name):
                st = wstage.tile([128, KT, kt_cols], F32, tag="wst",
                                 name="wst")
                nc.sync.dma_start(
                    out=st, in_=dram_ap.rearrange("(kt p) c -> p kt c", p=128))
                bf = persist.tile([128, KT, kt_cols], BF16, tag=name, name=name)
                nc.vector.tensor_copy(out=bf, in_=st)
                return bf

            wq_bf = load_cast(wq_t[:, :], DCOL, "wq")
            wk_bf = load_cast(wk_t[:, :], DCOL, "wk")
            wv_bf = load_cast(wv_t[:, :], DCOL, "wv")
            wo_bf = persist.tile([128, KT, D], BF16, tag="wo", name="wo")
            for kt in range(KT):
                st = wstage.tile([128, D], F32, tag="wost", name="wost")
                nc.sync.dma_start(out=st, in_=wo_t[kt * 128:(kt + 1) * 128, :])
                nc.vector.tensor_copy(out=wo_bf[:, kt, :], in_=st)
            bias_b = persist.tile([128, D], F32, tag="bias", name="bias_b")
            nc.sync.dma_start(out=bias_b, in_=bo_t[0:1, :].to_broadcast([128, D]))
            return wq_bf, wk_bf, wv_bf, wo_bf, bias_b

        def projections(b, xT, wq_bf, wk_bf, wv_bf):
            qT = persist.tile([128, S], BF16, tag=f"qT{b}", name=f"qT{b}")
            kTt = persist.tile([128, S], BF16, tag=f"kT{b}", name=f"kT{b}")
            for w_bf, dest in ((wq_bf, qT), (wk_bf, kTt)):
                for nt in range(NQS):
                    ps = ps_mm.tile([128, 2, 512], F32, tag="mm", name="ps")
                    for kt in range(KT):
                        nc.tensor.matmul(
                            ps[:, 0, :], lhsT=w_bf[:, kt, :],
                            rhs=xT[:, kt, nt * 512:(nt + 1) * 512],
                            start=(kt == 0), stop=(kt == KT - 1))
                    nc.vector.tensor_copy(
                        out=dest[:, nt * 512:(nt + 1) * 512], in_=ps[:, 0, :])
            # V' natural [s, 2*(64+1)] with ones columns
            vp = persist.tile([128, SB, H_PER * (DH + 1)], BF16,
                              tag=f"vp{b}", name=f"vp{b}")
            ones_view = vp.rearrange("p s (h c) -> p s h c", c=DH + 1)[:, :, :, DH:]
            nc.vector.memset(ones_view, 1.0)
            for sb in range(SB):
                ps = ps_mm.tile([128, 2, 512], F32, tag="mm", name="ps")
                for kt in range(KT):
                    nc.tensor.matmul(
                        ps[:, 0, 0:DCOL], lhsT=xT[:, kt, sb * 128:(sb + 1) * 128],
                        rhs=wv_bf[:, kt, :],
                        start=(kt == 0), stop=(kt == KT - 1))
                dst = vp.rearrange("p s (h c) -> p s h c", c=DH + 1)[:, sb, :, :DH]
                nc.vector.tensor_copy(
                    out=dst, in_=ps[:, 0, 0:DCOL].rearrange("p (h c) -> p h c", c=DH))
            return qT, kTt, vp

        # Attention, software-pipelined GLOBALLY across (batch, head, q-span)
        # in PAIRS of k-blocks: scores for a pair land in a 2-bank psum tile,
        # ONE exp covers both halves, and the AV matmuls trail DEPTH pairs
        # behind, so the PE stream has a single priming point for the whole
        # attention phase and never cools the HAM clock gate.
        DEPTH = 3

        def attention_steps(h, b, qT, kTt, vp, post_qs):
            """Yield (emit_scores, emit_av) closures per pair for one
            (head, batch); emit_av of the last pair runs the epilogue."""
            hr = h * DH
            for qs in range(NQS):
                nkb = 4 * qs + 4
                npair = nkb // 2
                o_ps = ps_o.tile([DH + 1, 512], F32, tag="o", name="o_ps")
                box = {}

                def mk_scores(pr, qs=qs, o_ps=o_ps, box=box):
                    def emit_scores():
                        s_ps = ps_mm.tile([128, 2, 512], F32, tag="mm",
                                          name="s_ps")
                        for i in range(2):
                            kb = 2 * pr + i
                            nc.tensor.matmul(
                                s_ps[:, i, :],
                                lhsT=kTt[hr:hr + DH, kb * 128:(kb + 1) * 128],
                                rhs=qT[hr:hr + DH, qs * 512:(qs + 1) * 512],
                                start=True, stop=True)
                        w_bf_t = wpool.tile([128, 2, 512], BF16, tag="w",
                                            name="w_bf_t")
                        diag = 2 * pr + 1 >= 4 * qs
                        if not diag:
                            nc.scalar.activation(
                                w_bf_t, s_ps, mybir.ActivationFunctionType.Exp,
                                scale=float(SCALE))
                        else:
                            # exp only the live columns; affine_select zeroes
                            # the rest (reading garbage, writing fill).
                            for i in range(2):
                                kb = 2 * pr + i
                                lo = max(0, 128 * (kb - 4 * qs))
                                nc.scalar.activation(
                                    w_bf_t[:, i, lo:512], s_ps[:, i, lo:512],
                                    mybir.ActivationFunctionType.Exp,
                                    scale=float(SCALE))
                        for i in range(2):
                            kb = 2 * pr + i
                            if kb >= 4 * qs:
                                # causal: keep iff (512qs+f) - (128kb+p) >= 0
                                nc.gpsimd.affine_select(
                                    out=w_bf_t[:, i, :], in_=w_bf_t[:, i, :],
                                    pattern=[[1, 512]],
                                    compare_op=mybir.AluOpType.is_ge,
                                    fill=0.0,
                                    base=512 * qs - 128 * kb,
                                    channel_multiplier=-1)
                        box[pr] = w_bf_t
                    return emit_scores

                def mk_av(pr, qs=qs, o_ps=o_ps, box=box, nkb=nkb,
                          npair=npair):
                    def emit_av():
                        for i in range(2):
                            kb = 2 * pr + i
                            nc.tensor.matmul(
                                o_ps,
                                lhsT=vp[:, kb, h * (DH + 1):(h + 1) * (DH + 1)],
                                rhs=box[pr][:, i, :],
                                start=(kb == 0), stop=(kb == nkb - 1))
                        del box[pr]
                        if pr == npair - 1:
                            denom = spool.tile([1, 512], F32, tag="den",
                                               name="denom")
                            nc.vector.tensor_copy(out=denom,
                                                  in_=o_ps[DH:DH + 1, :])
                            recip = spool.tile([1, 512], F32, tag="rec",
                                               name="recip")
                            nc.vector.reciprocal(recip, denom)
                            rb = rbpool.tile([DH, 512], F32, tag="rb",
                                             name="rb")
                            nc.gpsimd.partition_broadcast(rb, recip)
                            nc.vector.tensor_mul(
                                attnT[b][hr:hr + DH,
                                         qs * 512:(qs + 1) * 512],
                                o_ps[0:DH, :], rb)
                            if post_qs is not None:
                                post_qs(qs)
                    return emit_av

                for pr in range(npair):
                    yield mk_scores(pr), mk_av(pr)

        def run_attention_pipeline(blocks):
            """blocks: list of generators from attention_steps. Runs one
            DEPTH-deep pipeline across all of them."""
            steps = [st for blk in blocks for st in blk]
            n = len(steps)
            for i in range(n + DEPTH):
                if i < n:
                    steps[i][0]()          # scores/exp/mask for step i
                if i >= DEPTH:
                    steps[i - DEPTH][1]()  # AV for step i-DEPTH

        def exchange(h):
            """Ship head h's attn^T (both batches) and run its A2A."""
            hr = h * DH
            for b in range(B):
                nc.sync.dma_start(
                    out=a2a_in[h][4 * b:4 * (b + 1)].rearrange("j p c -> p j c"),
                    in_=attnT[b][hr:hr + DH, :].rearrange("p (j c) -> p j c",
                                                          c=512))
            nc.gpsimd.collective_compute(
                "AllToAll", mybir.AluOpType.bypass,
                replica_groups=[list(range(8))],
                ins=[a2a_in[h].ap().opt()], outs=[a2a_out[h].ap().opt()],
            )

        # ---- emission order ------------------------------------------------
        xT0 = persist.tile([128, KT, S], BF16, tag="xT0", name="xT0")
        load_transpose_x(0, xT0)
        wq_bf, wk_bf, wv_bf, wo_bf, bias_b = load_weights()
        qT0, kT0, vp0 = projections(0, xT0, wq_bf, wk_bf, wv_bf)
        xT1 = persist.tile([128, KT, S], BF16, tag="xT1", name="xT1")
        load_transpose_x(1, xT1)
        qT1, kT1, vp1 = projections(1, xT1, wq_bf, wk_bf, wv_bf)

        # head 0 both batches, then exchange(0) fires while head 1 computes
        def post_h0_b1(qs):
            if qs == NQS - 1:
                exchange(0)

        run_attention_pipeline([
            attention_steps(0, 0, qT0, kT0, vp0, None),
            attention_steps(0, 1, qT1, kT1, vp1, post_h0_b1),
            attention_steps(1, 0, qT0, kT0, vp0, None),
            attention_steps(1, 1, qT1, kT1, vp1, None),
        ])
        exchange(1)

        # ---- out_proj on this core's [1024, 512] gathered attn^T -----------
        # global row 128*kt + p: p<64 from head-0 exchange chunk kt, p>=64
        # from head-1 exchange chunk kt.
        g_sb = persist.tile([128, KT, 512], BF16, tag="g", name="g_sb")
        for h in range(H_PER):
            nc.sync.dma_start(
                out=g_sb[h * DH:(h + 1) * DH, :, :],
                in_=a2a_out[h].ap().rearrange("kt p c -> p kt c"))
        for sb in range(4):
            for nt in range(2):
                ps = ps_mm.tile([128, 2, 512], F32, tag="mm", name="ps")
                for kt in range(KT):
                    nc.tensor.matmul(
                        ps[:, 0, :], lhsT=g_sb[:, kt, sb * 128:(sb + 1) * 128],
                        rhs=wo_bf[:, kt, nt * 512:(nt + 1) * 512],
                        start=(kt == 0), stop=(kt == KT - 1))
                ot = opool.tile([128, 512], F32, tag="ot")
                ps = ps[:, 0, :]
                nc.vector.tensor_add(ot, ps, bias_b[:, nt * 512:(nt + 1) * 512])
                nc.sync.dma_start(
                    out=out_t[sb * 128:(sb + 1) * 128, nt * 512:(nt + 1) * 512],
                    in_=ot)

    nc.compile()
    return nc


def shard_inputs(x, Wq, Wk, Wv, Wo, bo):
    """Full inputs -> per-core in_maps."""
    x = np.ascontiguousarray(np.asarray(x, dtype=np.float32))
    Wq = np.asarray(Wq, dtype=np.float32)
    Wk = np.asarray(Wk, dtype=np.float32)
    Wv = np.asarray(Wv, dtype=np.float32)
    Wo = np.ascontiguousarray(np.asarray(Wo, dtype=np.float32))
    bo = np.asarray(bo, dtype=np.float32).reshape(1, D)
    in_maps = []
    for c in range(N_CORES):
        cols = slice(c * DCOL, (c + 1) * DCOL)
        in_maps.append({
            "x": x,
            "wq": np.ascontiguousarray(Wq[:, cols]),
            "wk": np.ascontiguousarray(Wk[:, cols]),
            "wv": np.ascontiguousarray(Wv[:, cols]),
            "wo": Wo,
            "bo": bo,
        })
    return in_maps


def assemble_output(results):
    """Per-core out slices -> full [B, S, D]."""
    out = np.empty((B, S, D), dtype=np.float32)
    for c in range(N_CORES):
        b, sl = c // 4, c % 4
        out[b, sl * S_SLICE:(sl + 1) * S_SLICE, :] = results[c]["out"]
    return out


def kernel(x, Wq, Wk, Wv, Wo, bo):
    if "nc" not in _CACHE:
        _CACHE["nc"] = build()
    nc = _CACHE["nc"]
    in_maps = shard_inputs(x, Wq, Wk, Wv, Wo, bo)
    res = run_bass_kernel_spmd(nc, in_maps, core_ids=list(range(N_CORES)))
    return assemble_output(res.results)


# revision 32
# speedup vs baseline: 1.0328x; 1.0328x over previous
"""Multi-head causal attention on 8 Trainium2 NeuronCores.

Problem: x [2, 2048, 1024] f32; Wq/Wk/Wv [1024, 1024]; Wo [1024, 1024]; bo [1024].
  q/k/v = split_heads(x @ W*)  (16 heads, head_dim 64)
  scores = q k^T, causal mask, / sqrt(1024), softmax, out = (w v) @ Wo + bo

Sharding: tensor-parallel over heads, 8-way (Megatron-style): core c computes
heads {2c, 2c+1} for BOTH batches. The concat+out_proj needs all heads, so
cores exchange attention outputs with a single 8-rank AllToAll (each core
sends, per destination core j, its two heads' attn^T restricted to j's output
row-slice). After the exchange core j holds attn^T [1024, 512] for
(batch j//4, rows 512*(j%4):+512), applies the full out_proj + bias, and
returns its 512-row slice of the output; the host reassembles.

On-chip layout trick: attention is computed fully transposed
(scores^T = K Q^T in [k, q] layout) so the softmax weights come out exactly in
the layout the attn-value matmul wants as its moving operand, and the AV
result comes out as attn^T [d, q] which is exactly the stationary layout
out_proj wants. Zero on-chip transposes in the attention path. The softmax
denominator is obtained for free by augmenting V with a ones-column (row 64 of
the AV psum accumulates sum(w)).

Compute dtype bf16 (fp32 accumulation in PSUM).
"""

from contextlib import ExitStack

import numpy as np

import concourse.bass as bass
import concourse.tile as tile
from concourse import bacc, mybir
from concourse.bass_utils import run_bass_kernel_spmd
from concourse.masks import make_identity

F32 = mybir.dt.float32
BF16 = mybir.dt.bfloat16

N_CORES = 8
B = 2
S = 2048
D = 1024
H = 16
DH = 64
H_PER = 2              # heads per core
DCOL = H_PER * DH      # 128: projection output cols per core
KT = D // 128          # 8 contraction tiles
SB = S // 128          # 16 sequence blocks
NQS = S // 512         # 4 q-spans
S_SLICE = S // 4       # 512 output rows per core
SCALE = 1.0 / np.sqrt(np.float32(D))

_CACHE = {}


def build():
    """Build the SPMD program (identical on all 8 cores)."""
    nc = bacc.Bacc("TRN2", target_bir_lowering=False, debug=False)

    x_t = nc.dram_tensor("x", [B, S, D], F32, kind="ExternalInput")
    wq_t = nc.dram_tensor("wq", [D, DCOL], F32, kind="ExternalInput")
    wk_t = nc.dram_tensor("wk", [D, DCOL], F32, kind="ExternalInput")
    wv_t = nc.dram_tensor("wv", [D, DCOL], F32, kind="ExternalInput")
    wo_t = nc.dram_tensor("wo", [D, D], F32, kind="ExternalInput")
    bo_t = nc.dram_tensor("bo", [1, D], F32, kind="ExternalInput")
    out_t = nc.dram_tensor("out", [S_SLICE, D], F32, kind="ExternalOutput")

    # collective buffers (internal DRAM); the attn^T exchange is split per
    # local head so the first A2A overlaps the second half of attention.
    warm_in = nc.dram_tensor("warm_in", [8, 16], F32)
    warm_out = nc.dram_tensor("warm_out", [8, 16], F32)
    a2a_in = [
        nc.dram_tensor(f"a2a_in{h}", [8, DH, 512], BF16) for h in range(H_PER)
    ]
    a2a_out = [
        nc.dram_tensor(f"a2a_out{h}", [8, DH, 512], BF16) for h in range(H_PER)
    ]

    with tile.TileContext(nc) as tc, ExitStack() as ctx:
        const = ctx.enter_context(tc.tile_pool(name="const", bufs=1))
        persist = ctx.enter_context(tc.tile_pool(name="persist", bufs=1))
        stage = ctx.enter_context(tc.tile_pool(name="stage", bufs=3))
        wstage = ctx.enter_context(tc.tile_pool(name="wstage", bufs=2))
        wpool = ctx.enter_context(tc.tile_pool(name="wpool", bufs=4))
        rbpool = ctx.enter_context(tc.tile_pool(name="rbpool", bufs=3))
        spool = ctx.enter_context(tc.tile_pool(name="spool", bufs=4))
        opool = ctx.enter_context(tc.tile_pool(name="opool", bufs=3))
        ps_mm = ctx.enter_context(tc.tile_pool(name="ps_mm", bufs=2, space="PSUM"))
        ps_o = ctx.enter_context(tc.tile_pool(name="ps_o", bufs=2, space="PSUM"))
        ps_t = ctx.enter_context(tc.tile_pool(name="ps_t", bufs=2, space="PSUM"))

        # ---- warmup collective: absorbs the per-execution ncfw entry cost
        # concurrently with the compute phase.
        nc.gpsimd.collective_compute(
            "AllToAll", mybir.AluOpType.bypass,
            replica_groups=[list(range(8))],
            ins=[warm_in.ap().opt()], outs=[warm_out.ap().opt()],
        )

        identity = const.tile([128, 128], BF16)
        make_identity(nc, identity)

        attnT = [
            persist.tile([128, S], BF16, tag=f"attnT{b}", name=f"attnT{b}")
            for b in range(B)
        ]

        # ---- phase helpers -------------------------------------------------
        def transpose_x_step(b, xT, sb):
            """One 128-row block of x[b] -> x^T bf16 columns, via PE."""
            xn = stage.tile([128, D], F32, tag="xn", name="xn")
            nc.sync.dma_start(out=xn, in_=x_t[b, sb * 128:(sb + 1) * 128, :])
            xb = stage.tile([128, D], BF16, tag="xb", name="xb")
            nc.vector.tensor_copy(out=xb, in_=xn)
            for g in range(2):  # 2 groups of 4 d-blocks -> one psum bank
                pt = ps_t.tile([128, 4, 128], BF16, tag="pt", name="pt")
                for k in range(4):
                    kt = g * 4 + k
                    nc.tensor.transpose(
                        pt[:, k, :], xb[:, kt * 128:(kt + 1) * 128], identity)
                dst = xT[:, g * 4:(g + 1) * 4, sb * 128:(sb + 1) * 128]
                nc.vector.tensor_copy(out=dst, in_=pt)

        def load_transpose_x(b, xT):
            for sb in range(SB):
                transpose_x_step(b, xT, sb)

        def load_weights():
            def load_cast(dram_ap, kt_cols, name):
                st = wstage.tile([128, KT, kt_cols], F32, tag="wst",
                                 name="wst")
                nc.sync.dma_start(
                    out=st, in_=dram_ap.rearrange("(kt p) c -> p kt c", p=128))
                bf = persist.tile([128, KT, kt_cols], BF16, tag=name, name=name)
                nc.vector.tensor_copy(out=bf, in_=st)
                return bf

            wq_bf = load_cast(wq_t[:, :], DCOL, "wq")
            wk_bf = load_cast(wk_t[:, :], DCOL, "wk")
            wv_bf = load_cast(wv_t[:, :], DCOL, "wv")
            wo_bf = persist.tile([128, KT, D], BF16, tag="wo", name="wo")
            for kt in range(KT):
                st = wstage.tile([128, D], F32, tag="wost", name="wost")
                nc.sync.dma_start(out=st, in_=wo_t[kt * 128:(kt + 1) * 128, :])
                nc.vector.tensor_copy(out=wo_bf[:, kt, :], in_=st)
            bias_b = persist.tile([128, D], F32, tag="bias", name="bias_b")
            nc.sync.dma_start(out=bias_b, in_=bo_t[0:1, :].to_broadcast([128, D]))
            return wq_bf, wk_bf, wv_bf, wo_bf, bias_b

        def alloc_proj_tiles(b):
            qT = persist.tile([128, S], BF16, tag=f"qT{b}", name=f"qT{b}")
            kTt = persist.tile([128, S], BF16, tag=f"kT{b}", name=f"kT{b}")
            vp = persist.tile([128, SB, H_PER * (DH + 1)], BF16,
                              tag=f"vp{b}", name=f"vp{b}")
            return qT, kTt, vp

        def proj_qk_step(xT, w_bf, dest, nt):
            ps = ps_mm.tile([128, 2, 512], F32, tag="mm", name="ps")
            for kt in range(KT):
                nc.tensor.matmul(
                    ps[:, 0, :], lhsT=w_bf[:, kt, :],
                    rhs=xT[:, kt, nt * 512:(nt + 1) * 512],
                    start=(kt == 0), stop=(kt == KT - 1))
            nc.vector.tensor_copy(
                out=dest[:, nt * 512:(nt + 1) * 512], in_=ps[:, 0, :])

        def proj_v_step(xT, wv_bf, vp, sb):
            ps = ps_mm.tile([128, 2, 512], F32, tag="mm", name="ps")
            for kt in range(KT):
                nc.tensor.matmul(
                    ps[:, 0, 0:DCOL], lhsT=xT[:, kt, sb * 128:(sb + 1) * 128],
                    rhs=wv_bf[:, kt, :],
                    start=(kt == 0), stop=(kt == KT - 1))
            dst = vp.rearrange("p s (h c) -> p s h c", c=DH + 1)[:, sb, :, :DH]
            nc.vector.tensor_copy(
                out=dst, in_=ps[:, 0, 0:DCOL].rearrange("p (h c) -> p h c", c=DH))

        def projections(b, xT, qT, kTt, vp, wq_bf, wk_bf, wv_bf):
            for w_bf, dest in ((wq_bf, qT), (wk_bf, kTt)):
                for nt in range(NQS):
                    proj_qk_step(xT, w_bf, dest, nt)
            ones_view = vp.rearrange("p s (h c) -> p s h c", c=DH + 1)[:, :, :, DH:]
            nc.vector.memset(ones_view, 1.0)
            for sb in range(SB):
                proj_v_step(xT, wv_bf, vp, sb)

        # Attention, software-pipelined GLOBALLY across (batch, head, q-span)
        # in PAIRS of k-blocks: scores for a pair land in a 2-bank psum tile,
        # ONE exp covers both halves, and the AV matmuls trail DEPTH pairs
        # behind, so the PE stream has a single priming point for the whole
        # attention phase and never cools the HAM clock gate.
        DEPTH = 3

        def attention_steps(h, b, qT, kTt, vp, post_qs):
            """Yield (emit_scores, emit_av) closures per pair for one
            (head, batch); emit_av of the last pair runs the epilogue."""
            hr = h * DH
            for qs in range(NQS):
                nkb = 4 * qs + 4
                npair = nkb // 2
                o_ps = ps_o.tile([DH + 1, 512], F32, tag="o", name="o_ps")
                box = {}

                def mk_scores(pr, qs=qs, o_ps=o_ps, box=box):
                    def emit_scores():
                        s_ps = ps_mm.tile([128, 2, 512], F32, tag="mm",
                                          name="s_ps")
                        for i in range(2):
                            kb = 2 * pr + i
                            nc.tensor.matmul(
                                s_ps[:, i, :],
                                lhsT=kTt[hr:hr + DH, kb * 128:(kb + 1) * 128],
                                rhs=qT[hr:hr + DH, qs * 512:(qs + 1) * 512],
                                start=True, stop=True)
                        w_bf_t = wpool.tile([128, 2, 512], BF16, tag="w",
                                            name="w_bf_t")
                        diag = 2 * pr + 1 >= 4 * qs
                        if not diag:
                            nc.scalar.activation(
                                w_bf_t, s_ps, mybir.ActivationFunctionType.Exp,
                                scale=float(SCALE))
                        else:
                            # exp only the live columns; affine_select zeroes
                            # the rest (reading garbage, writing fill).
                            for i in range(2):
                                kb = 2 * pr + i
                                lo = max(0, 128 * (kb - 4 * qs))
                                nc.scalar.activation(
                                    w_bf_t[:, i, lo:512], s_ps[:, i, lo:512],
                                    mybir.ActivationFunctionType.Exp,
                                    scale=float(SCALE))
                        for i in range(2):
                            kb = 2 * pr + i
                            if kb >= 4 * qs:
                                # causal: keep iff (512qs+f) - (128kb+p) >= 0
                                nc.gpsimd.affine_select(
                                    out=w_bf_t[:, i, :], in_=w_bf_t[:, i, :],
                                    pattern=[[1, 512]],
                                    compare_op=mybir.AluOpType.is_ge,
                                    fill=0.0,
                                    base=512 * qs - 128 * kb,
                                    channel_multiplier=-1)
                        box[pr] = w_bf_t
                    return emit_scores

                def mk_av(pr, qs=qs, o_ps=o_ps, box=box, nkb=nkb,
                          npair=npair):
                    def emit_av():
                        for i in range(2):
                            kb = 2 * pr + i
                            nc.tensor.matmul(
                                o_ps,
                                lhsT=vp[:, kb, h * (DH + 1):(h + 1) * (DH + 1)],
                                rhs=box[pr][:, i, :],
                                start=(kb == 0), stop=(kb == nkb - 1))
                        del box[pr]
                        if pr == npair - 1:
                            denom = spool.tile([1, 512], F32, tag="den",
                                               name="denom")
                            nc.vector.tensor_copy(out=denom,
                                                  in_=o_ps[DH:DH + 1, :])
                            recip = spool.tile([1, 512], F32, tag="rec",
                                               name="recip")
                            nc.vector.reciprocal(recip, denom)
                            rb = rbpool.tile([DH, 512], F32, tag="rb",
                                             name="rb")
                            nc.gpsimd.partition_broadcast(rb, recip)
                            nc.vector.tensor_mul(
                                attnT[b][hr:hr + DH,
                                         qs * 512:(qs + 1) * 512],
                                o_ps[0:DH, :], rb)
                            if post_qs is not None:
                                post_qs(qs)
                    return emit_av

                for pr in range(npair):
                    yield mk_scores(pr), mk_av(pr)

        def run_attention_pipeline(blocks, fillers=(), inject_every=2):
            """blocks: list of generators from attention_steps. Runs one
            DEPTH-deep pipeline across all of them, injecting one filler
            thunk (extra PE work) every `inject_every` steps to keep the
            PE fully busy while ACT works through the exp chain."""
            steps = [st for blk in blocks for st in blk]
            n = len(steps)
            fi = 0
            fillers = list(fillers)
            # all fillers must be emitted within the first `span` steps so
            # that later blocks (which consume the fillers' outputs) are
            # emitted after them.
            span = max(1, n // 2 - DEPTH)
            per_step = -(-len(fillers) // span) if fillers else 0
            for i in range(n + DEPTH):
                if i < n:
                    steps[i][0]()          # scores/exp/mask for step i
                for _ in range(per_step):
                    if fi < len(fillers) and i < span:
                        fillers[fi]()
                        fi += 1
                if i >= DEPTH:
                    steps[i - DEPTH][1]()  # AV for step i-DEPTH
            while fi < len(fillers):
                fillers[fi]()
                fi += 1

        def exchange(h):
            """Ship head h's attn^T (both batches) and run its A2A."""
            hr = h * DH
            for b in range(B):
                nc.sync.dma_start(
                    out=a2a_in[h][4 * b:4 * (b + 1)].rearrange("j p c -> p j c"),
                    in_=attnT[b][hr:hr + DH, :].rearrange("p (j c) -> p j c",
                                                          c=512))
            nc.gpsimd.collective_compute(
                "AllToAll", mybir.AluOpType.bypass,
                replica_groups=[list(range(8))],
                ins=[a2a_in[h].ap().opt()], outs=[a2a_out[h].ap().opt()],
            )

        # ---- emission order ------------------------------------------------
        xT0 = persist.tile([128, KT, S], BF16, tag="xT0", name="xT0")
        load_transpose_x(0, xT0)
        wq_bf, wk_bf, wv_bf, wo_bf, bias_b = load_weights()
        qT0, kT0, vp0 = alloc_proj_tiles(0)
        projections(0, xT0, qT0, kT0, vp0, wq_bf, wk_bf, wv_bf)

        # batch 1's x-transpose + projections become PE filler inside the
        # batch-0 attention pipeline (they keep PE at 100% duty while ACT
        # works through the exp chain, so the HAM clock gate stays open).
        xT1 = persist.tile([128, KT, S], BF16, tag="xT1", name="xT1")
        qT1, kT1, vp1 = alloc_proj_tiles(1)
        ones_view1 = vp1.rearrange("p s (h c) -> p s h c",
                                   c=DH + 1)[:, :, :, DH:]
        nc.vector.memset(ones_view1, 1.0)
        fillers = []
        for sb in range(SB):
            fillers.append(lambda sb=sb: transpose_x_step(1, xT1, sb))
        for w_bf, dest in ((wq_bf, qT1), (wk_bf, kT1)):
            for nt in range(NQS):
                fillers.append(
                    lambda w_bf=w_bf, dest=dest, nt=nt:
                        proj_qk_step(xT1, w_bf, dest, nt))
        for sb in range(SB):
            fillers.append(lambda sb=sb: proj_v_step(xT1, wv_bf, vp1, sb))

        # batch-0 heads first (their inputs are ready); batch-1 heads last,
        # by which point the filler projections have completed.
        # exchange(0) = head 0 of both batches, fires while (h1,b1) computes.
        def post_h0_b1(qs):
            if qs == NQS - 1:
                exchange(0)

        run_attention_pipeline(
            [
                attention_steps(0, 0, qT0, kT0, vp0, None),
                attention_steps(1, 0, qT0, kT0, vp0, None),
                attention_steps(0, 1, qT1, kT1, vp1, post_h0_b1),
                attention_steps(1, 1, qT1, kT1, vp1, None),
            ],
            fillers=fillers,
            inject_every=2,
        )
        exchange(1)

        # ---- out_proj on this core's [1024, 512] gathered attn^T -----------
        # global row 128*kt + p: p<64 from head-0 exchange chunk kt, p>=64
        # from head-1 exchange chunk kt.
        g_sb = persist.tile([128, KT, 512], BF16, tag="g", name="g_sb")
        for h in range(H_PER):
            nc.sync.dma_start(
                out=g_sb[h * DH:(h + 1) * DH, :, :],
                in_=a2a_out[h].ap().rearrange("kt p c -> p kt c"))
        for sb in range(4):
            for nt in range(2):
                ps = ps_mm.tile([128, 2, 512], F32, tag="mm", name="ps")
                for kt in range(KT):
                    nc.tensor.matmul(
                        ps[:, 0, :], lhsT=g_sb[:, kt, sb * 128:(sb + 1) * 128],
                        rhs=wo_bf[:, kt, nt * 512:(nt + 1) * 512],
                        start=(kt == 0), stop=(kt == KT - 1))
                ot = opool.tile([128, 512], F32, tag="ot")
                ps = ps[:, 0, :]
                nc.vector.tensor_add(ot, ps, bias_b[:, nt * 512:(nt + 1) * 512])
                nc.sync.dma_start(
                    out=out_t[sb * 128:(sb + 1) * 128, nt * 512:(nt + 1) * 512],
                    in_=ot)

    nc.compile()
    return nc


def shard_inputs(x, Wq, Wk, Wv, Wo, bo):
    """Full inputs -> per-core in_maps."""
    x = np.ascontiguousarray(np.asarray(x, dtype=np.float32))
    Wq = np.asarray(Wq, dtype=np.float32)
    Wk = np.asarray(Wk, dtype=np.float32)
    Wv = np.asarray(Wv, dtype=np.float32)
    Wo = np.ascontiguousarray(np.asarray(Wo, dtype=np.float32))
    bo = np.asarray(bo, dtype=np.float32).reshape(1, D)
    in_maps = []
    for c in range(N_CORES):
        cols = slice(c * DCOL, (c + 1) * DCOL)
        in_maps.append({
            "x": x,
            "wq": np.ascontiguousarray(Wq[:, cols]),
            "wk": np.ascontiguousarray(Wk[:, cols]),
            "wv": np.ascontiguousarray(Wv[:, cols]),
            "wo": Wo,
            "bo": bo,
        })
    return in_maps


def assemble_output(results):
    """Per-core out slices -> full [B, S, D]."""
    out = np.empty((B, S, D), dtype=np.float32)
    for c in range(N_CORES):
        b, sl = c // 4, c % 4
        out[b, sl * S_SLICE:(sl + 1) * S_SLICE, :] = results[c]["out"]
    return out


def kernel(x, Wq, Wk, Wv, Wo, bo):
    if "nc" not in _CACHE:
        _CACHE["nc"] = build()
    nc = _CACHE["nc"]
    in_maps = shard_inputs(x, Wq, Wk, Wv, Wo, bo)
    res = run_bass_kernel_spmd(nc, in_maps, core_ids=list(range(N_CORES)))
    return assemble_output(res.results)


# revision 36
# speedup vs baseline: 1.0839x; 1.0495x over previous
"""Multi-head causal attention on 8 Trainium2 NeuronCores.

Problem: x [2, 2048, 1024] f32; Wq/Wk/Wv [1024, 1024]; Wo [1024, 1024]; bo [1024].
  q/k/v = split_heads(x @ W*)  (16 heads, head_dim 64)
  scores = q k^T, causal mask, / sqrt(1024), softmax, out = (w v) @ Wo + bo

Sharding: tensor-parallel over heads, 8-way (Megatron-style): core c computes
heads {2c, 2c+1} for BOTH batches. The concat+out_proj needs all heads, so
cores exchange attention outputs with a single 8-rank AllToAll (each core
sends, per destination core j, its two heads' attn^T restricted to j's output
row-slice). After the exchange core j holds attn^T [1024, 512] for
(batch j//4, rows 512*(j%4):+512), applies the full out_proj + bias, and
returns its 512-row slice of the output; the host reassembles.

On-chip layout trick: attention is computed fully transposed
(scores^T = K Q^T in [k, q] layout) so the softmax weights come out exactly in
the layout the attn-value matmul wants as its moving operand, and the AV
result comes out as attn^T [d, q] which is exactly the stationary layout
out_proj wants. Zero on-chip transposes in the attention path. The softmax
denominator is obtained for free by augmenting V with a ones-column (row 64 of
the AV psum accumulates sum(w)).

Compute dtype bf16 (fp32 accumulation in PSUM).
"""

from contextlib import ExitStack

import numpy as np

import concourse.bass as bass
import concourse.tile as tile
from concourse import bacc, mybir
from concourse.bass_utils import run_bass_kernel_spmd
from concourse.masks import make_identity

F32 = mybir.dt.float32
BF16 = mybir.dt.bfloat16

N_CORES = 8
B = 2
S = 2048
D = 1024
H = 16
DH = 64
H_PER = 2              # heads per core
DCOL = H_PER * DH      # 128: projection output cols per core
KT = D // 128          # 8 contraction tiles
SB = S // 128          # 16 sequence blocks
NQS = S // 512         # 4 q-spans
S_SLICE = S // 4       # 512 output rows per core
SCALE = 1.0 / np.sqrt(np.float32(D))

_CACHE = {}


def build():
    """Build the SPMD program (identical on all 8 cores)."""
    nc = bacc.Bacc("TRN2", target_bir_lowering=False, debug=False)

    x_t = nc.dram_tensor("x", [B, S, D], F32, kind="ExternalInput")
    wq_t = nc.dram_tensor("wq", [D, DCOL], F32, kind="ExternalInput")
    wk_t = nc.dram_tensor("wk", [D, DCOL], F32, kind="ExternalInput")
    wv_t = nc.dram_tensor("wv", [D, DCOL], F32, kind="ExternalInput")
    wo_t = nc.dram_tensor("wo", [D, D], F32, kind="ExternalInput")
    bo_t = nc.dram_tensor("bo", [1, D], F32, kind="ExternalInput")
    out_t = nc.dram_tensor("out", [S_SLICE, D], F32, kind="ExternalOutput")

    # collective buffers (internal DRAM); the attn^T exchange is split per
    # local head so the first A2A overlaps the second half of attention.
    warm_in = nc.dram_tensor("warm_in", [8, 16], F32)
    warm_out = nc.dram_tensor("warm_out", [8, 16], F32)
    a2a_in = [
        nc.dram_tensor(f"a2a_in{h}", [8, DH, 512], BF16) for h in range(H_PER)
    ]
    a2a_out = [
        nc.dram_tensor(f"a2a_out{h}", [8, DH, 512], BF16) for h in range(H_PER)
    ]

    with tile.TileContext(nc) as tc, ExitStack() as ctx:
        const = ctx.enter_context(tc.tile_pool(name="const", bufs=1))
        persist = ctx.enter_context(tc.tile_pool(name="persist", bufs=1))
        stage = ctx.enter_context(tc.tile_pool(name="stage", bufs=3))
        wstage = ctx.enter_context(tc.tile_pool(name="wstage", bufs=1))
        wpool = ctx.enter_context(tc.tile_pool(name="wpool", bufs=4))
        rbpool = ctx.enter_context(tc.tile_pool(name="rbpool", bufs=2))
        spool = ctx.enter_context(tc.tile_pool(name="spool", bufs=2))
        opool = ctx.enter_context(tc.tile_pool(name="opool", bufs=3))
        ps_mm = ctx.enter_context(tc.tile_pool(name="ps_mm", bufs=2, space="PSUM"))
        ps_o = ctx.enter_context(tc.tile_pool(name="ps_o", bufs=2, space="PSUM"))
        ps_t = ctx.enter_context(tc.tile_pool(name="ps_t", bufs=2, space="PSUM"))

        # ---- warmup collective: absorbs the per-execution ncfw entry cost
        # concurrently with the compute phase.
        nc.gpsimd.collective_compute(
            "AllToAll", mybir.AluOpType.bypass,
            replica_groups=[list(range(8))],
            ins=[warm_in.ap().opt()], outs=[warm_out.ap().opt()],
        )

        identity = const.tile([128, 128], BF16)
        make_identity(nc, identity)

        attnT = [
            persist.tile([128, S], BF16, tag=f"attnT{b}", name=f"attnT{b}")
            for b in range(B)
        ]

        # ---- phase helpers -------------------------------------------------
        def transpose_x_step(b, xT, sb):
            """One 128-row block of x[b] -> x^T bf16 columns, via PE."""
            xn = stage.tile([128, D], F32, tag="xn", name="xn")
            nc.sync.dma_start(out=xn, in_=x_t[b, sb * 128:(sb + 1) * 128, :])
            xb = stage.tile([128, D], BF16, tag="xb", name="xb")
            nc.vector.tensor_copy(out=xb, in_=xn)
            for g in range(2):  # 2 groups of 4 d-blocks -> one psum bank
                pt = ps_t.tile([128, 4, 128], BF16, tag="pt", name="pt")
                for k in range(4):
                    kt = g * 4 + k
                    nc.tensor.transpose(
                        pt[:, k, :], xb[:, kt * 128:(kt + 1) * 128], identity)
                dst = xT[:, g * 4:(g + 1) * 4, sb * 128:(sb + 1) * 128]
                nc.vector.tensor_copy(out=dst, in_=pt)

        def load_transpose_x(b, xT):
            for sb in range(SB):
                transpose_x_step(b, xT, sb)

        def load_weights():
            def load_cast(dram_ap, kt_cols, name):
                st = wstage.tile([128, KT, kt_cols], F32, tag="wst",
                                 name="wst")
                nc.sync.dma_start(
                    out=st, in_=dram_ap.rearrange("(kt p) c -> p kt c", p=128))
                bf = persist.tile([128, KT, kt_cols], BF16, tag=name, name=name)
                nc.vector.tensor_copy(out=bf, in_=st)
                return bf

            wq_bf = load_cast(wq_t[:, :], DCOL, "wq")
            wk_bf = load_cast(wk_t[:, :], DCOL, "wk")
            wv_bf = load_cast(wv_t[:, :], DCOL, "wv")
            wo_bf = persist.tile([128, KT, D], BF16, tag="wo", name="wo")
            for kt in range(KT):
                st = wstage.tile([128, D], F32, tag="wost", name="wost")
                nc.sync.dma_start(out=st, in_=wo_t[kt * 128:(kt + 1) * 128, :])
                nc.vector.tensor_copy(out=wo_bf[:, kt, :], in_=st)
            bias_b = persist.tile([128, D], F32, tag="bias", name="bias_b")
            nc.sync.dma_start(out=bias_b, in_=bo_t[0:1, :].to_broadcast([128, D]))
            return wq_bf, wk_bf, wv_bf, wo_bf, bias_b

        def alloc_proj_tiles(b):
            # Q^T / K^T are stored per-head, zero-padded to 128 partitions
            # (rows 64:128 = 0) because matmuls with contraction dim <= 64
            # run at HALF rate on the PE.
            qT = [persist.tile([128, S], BF16, tag=f"qT{b}{h}",
                               name=f"qT{b}{h}") for h in range(H_PER)]
            kTt = [persist.tile([128, S], BF16, tag=f"kT{b}{h}",
                                name=f"kT{b}{h}") for h in range(H_PER)]
            vp = persist.tile([128, SB, H_PER * (DH + 1)], BF16,
                              tag=f"vp{b}", name=f"vp{b}")
            for t in qT + kTt:
                nc.vector.memset(t[DH:128, :], 0.0)
            return qT, kTt, vp

        def proj_qk_step(xT, w_bf, dest, nt):
            ps = ps_mm.tile([128, 2, 512], F32, tag="mm", name="ps")
            for kt in range(KT):
                nc.tensor.matmul(
                    ps[:, 0, :], lhsT=w_bf[:, kt, :],
                    rhs=xT[:, kt, nt * 512:(nt + 1) * 512],
                    start=(kt == 0), stop=(kt == KT - 1))
            for h in range(H_PER):
                nc.vector.tensor_copy(
                    out=dest[h][0:DH, nt * 512:(nt + 1) * 512],
                    in_=ps[h * DH:(h + 1) * DH, 0, :])

        def proj_v_step(xT, wv_bf, vp, sb):
            ps = ps_mm.tile([128, 2, 512], F32, tag="mm", name="ps")
            for kt in range(KT):
                nc.tensor.matmul(
                    ps[:, 0, 0:DCOL], lhsT=xT[:, kt, sb * 128:(sb + 1) * 128],
                    rhs=wv_bf[:, kt, :],
                    start=(kt == 0), stop=(kt == KT - 1))
            dst = vp.rearrange("p s (h c) -> p s h c", c=DH + 1)[:, sb, :, :DH]
            nc.vector.tensor_copy(
                out=dst, in_=ps[:, 0, 0:DCOL].rearrange("p (h c) -> p h c", c=DH))

        def projections(b, xT, qT, kTt, vp, wq_bf, wk_bf, wv_bf):
            for w_bf, dest in ((wq_bf, qT), (wk_bf, kTt)):
                for nt in range(NQS):
                    proj_qk_step(xT, w_bf, dest, nt)
            ones_view = vp.rearrange("p s (h c) -> p s h c", c=DH + 1)[:, :, :, DH:]
            nc.vector.memset(ones_view, 1.0)
            for sb in range(SB):
                proj_v_step(xT, wv_bf, vp, sb)

        # Attention, software-pipelined GLOBALLY across (batch, head, q-span)
        # in PAIRS of k-blocks: scores for a pair land in a 2-bank psum tile,
        # ONE exp covers both halves, and the AV matmuls trail DEPTH pairs
        # behind, so the PE stream has a single priming point for the whole
        # attention phase and never cools the HAM clock gate.
        DEPTH = 3

        def attention_steps(h, b, qT, kTt, vp, post_qs):
            """Yield (emit_scores, emit_av) closures per pair for one
            (head, batch); emit_av of the last pair runs the epilogue.
            qT/kTt are the per-head zero-padded tiles."""
            qTh, kTh = qT[h], kTt[h]
            hr = h * DH
            for qs in range(NQS):
                nkb = 4 * qs + 4
                npair = nkb // 2
                o_ps = ps_o.tile([DH + 1, 512], F32, tag="o", name="o_ps")
                box = {}

                def mk_scores(pr, qs=qs, o_ps=o_ps, box=box):
                    def emit_scores():
                        s_ps = ps_mm.tile([128, 2, 512], F32, tag="mm",
                                          name="s_ps")
                        for i in range(2):
                            kb = 2 * pr + i
                            nc.tensor.matmul(
                                s_ps[:, i, :],
                                lhsT=kTh[:, kb * 128:(kb + 1) * 128],
                                rhs=qTh[:, qs * 512:(qs + 1) * 512],
                                start=True, stop=True)
                        w_bf_t = wpool.tile([128, 2, 512], BF16, tag="w",
                                            name="w_bf_t")
                        diag = 2 * pr + 1 >= 4 * qs
                        if not diag:
                            nc.scalar.activation(
                                w_bf_t, s_ps, mybir.ActivationFunctionType.Exp,
                                scale=float(SCALE))
                        else:
                            # exp only the live columns; affine_select zeroes
                            # the rest (reading garbage, writing fill).
                            for i in range(2):
                                kb = 2 * pr + i
                                lo = max(0, 128 * (kb - 4 * qs))
                                nc.scalar.activation(
                                    w_bf_t[:, i, lo:512], s_ps[:, i, lo:512],
                                    mybir.ActivationFunctionType.Exp,
                                    scale=float(SCALE))
                        for i in range(2):
                            kb = 2 * pr + i
                            if kb >= 4 * qs:
                                # causal: keep iff (512qs+f) - (128kb+p) >= 0
                                nc.gpsimd.affine_select(
                                    out=w_bf_t[:, i, :], in_=w_bf_t[:, i, :],
                                    pattern=[[1, 512]],
                                    compare_op=mybir.AluOpType.is_ge,
                                    fill=0.0,
                                    base=512 * qs - 128 * kb,
                                    channel_multiplier=-1)
                        box[pr] = w_bf_t
                    return emit_scores

                def mk_av(pr, qs=qs, o_ps=o_ps, box=box, nkb=nkb,
                          npair=npair):
                    def emit_av():
                        for i in range(2):
                            kb = 2 * pr + i
                            nc.tensor.matmul(
                                o_ps,
                                lhsT=vp[:, kb, h * (DH + 1):(h + 1) * (DH + 1)],
                                rhs=box[pr][:, i, :],
                                start=(kb == 0), stop=(kb == nkb - 1))
                        del box[pr]
                        if pr == npair - 1:
                            denom = spool.tile([1, 512], F32, tag="den",
                                               name="denom")
                            nc.vector.tensor_copy(out=denom,
                                                  in_=o_ps[DH:DH + 1, :])
                            recip = spool.tile([1, 512], F32, tag="rec",
                                               name="recip")
                            nc.vector.reciprocal(recip, denom)
                            rb = rbpool.tile([DH, 512], F32, tag="rb",
                                             name="rb")
                            nc.gpsimd.partition_broadcast(rb, recip)
                            nc.vector.tensor_mul(
                                attnT[b][hr:hr + DH,
                                         qs * 512:(qs + 1) * 512],
                                o_ps[0:DH, :], rb)
                            if post_qs is not None:
                                post_qs(qs)
                    return emit_av

                for pr in range(npair):
                    yield mk_scores(pr), mk_av(pr)

        def run_attention_pipeline(blocks, fillers=(), inject_every=2):
            """blocks: list of generators from attention_steps. Runs one
            DEPTH-deep pipeline across all of them, injecting one filler
            thunk (extra PE work) every `inject_every` steps to keep the
            PE fully busy while ACT works through the exp chain."""
            steps = [st for blk in blocks for st in blk]
            n = len(steps)
            fi = 0
            fillers = list(fillers)
            # all fillers must be emitted within the first `span` steps so
            # that later blocks (which consume the fillers' outputs) are
            # emitted after them.
            span = max(1, n // 2 - DEPTH)
            per_step = -(-len(fillers) // span) if fillers else 0
            for i in range(n + DEPTH):
                if i < n:
                    steps[i][0]()          # scores/exp/mask for step i
                for _ in range(per_step):
                    if fi < len(fillers) and i < span:
                        fillers[fi]()
                        fi += 1
                if i >= DEPTH:
                    steps[i - DEPTH][1]()  # AV for step i-DEPTH
            while fi < len(fillers):
                fillers[fi]()
                fi += 1

        def exchange(h):
            """Ship head h's attn^T (both batches) and run its A2A."""
            hr = h * DH
            for b in range(B):
                nc.sync.dma_start(
                    out=a2a_in[h][4 * b:4 * (b + 1)].rearrange("j p c -> p j c"),
                    in_=attnT[b][hr:hr + DH, :].rearrange("p (j c) -> p j c",
                                                          c=512))
            nc.gpsimd.collective_compute(
                "AllToAll", mybir.AluOpType.bypass,
                replica_groups=[list(range(8))],
                ins=[a2a_in[h].ap().opt()], outs=[a2a_out[h].ap().opt()],
            )

        # ---- emission order ------------------------------------------------
        xT0 = persist.tile([128, KT, S], BF16, tag="xT", name="xT0")
        load_transpose_x(0, xT0)
        wq_bf, wk_bf, wv_bf, wo_bf, bias_b = load_weights()
        qT0, kT0, vp0 = alloc_proj_tiles(0)
        projections(0, xT0, qT0, kT0, vp0, wq_bf, wk_bf, wv_bf)

        # batch 1's x-transpose + projections become PE filler inside the
        # batch-0 attention pipeline (they keep PE at 100% duty while ACT
        # works through the exp chain, so the HAM clock gate stays open).
        xT1 = persist.tile([128, KT, S], BF16, tag="xT", name="xT1")
        qT1, kT1, vp1 = alloc_proj_tiles(1)
        ones_view1 = vp1.rearrange("p s (h c) -> p s h c",
                                   c=DH + 1)[:, :, :, DH:]
        nc.vector.memset(ones_view1, 1.0)
        fillers = []
        for sb in range(SB):
            fillers.append(lambda sb=sb: transpose_x_step(1, xT1, sb))
        for w_bf, dest in ((wq_bf, qT1), (wk_bf, kT1)):
            for nt in range(NQS):
                fillers.append(
                    lambda w_bf=w_bf, dest=dest, nt=nt:
                        proj_qk_step(xT1, w_bf, dest, nt))
        for sb in range(SB):
            fillers.append(lambda sb=sb: proj_v_step(xT1, wv_bf, vp1, sb))

        # batch-0 heads first (their inputs are ready); batch-1 heads last,
        # by which point the filler projections have completed.
        # exchange(0) = head 0 of both batches, fires while (h1,b1) computes.
        def post_h0_b1(qs):
            if qs == NQS - 1:
                exchange(0)

        run_attention_pipeline(
            [
                attention_steps(0, 0, qT0, kT0, vp0, None),
                attention_steps(1, 0, qT0, kT0, vp0, None),
                attention_steps(0, 1, qT1, kT1, vp1, post_h0_b1),
                attention_steps(1, 1, qT1, kT1, vp1, None),
            ],
            fillers=fillers,
            inject_every=2,
        )
        exchange(1)

        # ---- out_proj on this core's [1024, 512] gathered attn^T -----------
        # global row 128*kt + p: p<64 from head-0 exchange chunk kt, p>=64
        # from head-1 exchange chunk kt.
        g_sb = persist.tile([128, KT, 512], BF16, tag="g", name="g_sb")
        for h in range(H_PER):
            nc.sync.dma_start(
                out=g_sb[h * DH:(h + 1) * DH, :, :],
                in_=a2a_out[h].ap().rearrange("kt p c -> p kt c"))
        for sb in range(4):
            for nt in range(2):
                ps = ps_mm.tile([128, 2, 512], F32, tag="mm", name="ps")
                for kt in range(KT):
                    nc.tensor.matmul(
                        ps[:, 0, :], lhsT=g_sb[:, kt, sb * 128:(sb + 1) * 128],
                        rhs=wo_bf[:, kt, nt * 512:(nt + 1) * 512],
                        start=(kt == 0), stop=(kt == KT - 1))
                ot = opool.tile([128, 512], F32, tag="ot")
                ps = ps[:, 0, :]
                nc.vector.tensor_add(ot, ps, bias_b[:, nt * 512:(nt + 1) * 512])
                nc.sync.dma_start(
                    out=out_t[sb * 128:(sb + 1) * 128, nt * 512:(nt + 1) * 512],
                    in_=ot)

    nc.compile()
    return nc


def shard_inputs(x, Wq, Wk, Wv, Wo, bo):
    """Full inputs -> per-core in_maps."""
    x = np.ascontiguousarray(np.asarray(x, dtype=np.float32))
    Wq = np.asarray(Wq, dtype=np.float32)
    Wk = np.asarray(Wk, dtype=np.float32)
    Wv = np.asarray(Wv, dtype=np.float32)
    Wo = np.ascontiguousarray(np.asarray(Wo, dtype=np.float32))
    bo = np.asarray(bo, dtype=np.float32).reshape(1, D)
    in_maps = []
    for c in range(N_CORES):
        cols = slice(c * DCOL, (c + 1) * DCOL)
        in_maps.append({
            "x": x,
            "wq": np.ascontiguousarray(Wq[:, cols]),
            "wk": np.ascontiguousarray(Wk[:, cols]),
            "wv": np.ascontiguousarray(Wv[:, cols]),
            "wo": Wo,
            "bo": bo,
        })
    return in_maps


def assemble_output(results):
    """Per-core out slices -> full [B, S, D]."""
    out = np.empty((B, S, D), dtype=np.float32)
    for c in range(N_CORES):
        b, sl = c // 4, c % 4
        out[b, sl * S_SLICE:(sl + 1) * S_SLICE, :] = results[c]["out"]
    return out


def kernel(x, Wq, Wk, Wv, Wo, bo):
    if "nc" not in _CACHE:
        _CACHE["nc"] = build()
    nc = _CACHE["nc"]
    in_maps = shard_inputs(x, Wq, Wk, Wv, Wo, bo)
    res = run_bass_kernel_spmd(nc, in_maps, core_ids=list(range(N_CORES)))
    return assemble_output(res.results)
